# revision 3
# baseline (speedup 1.0000x reference)
"""Trainium2 Bass kernel for nn_Attention (B=4, N=2048, C=1024, H=16).

Sharding: 8 cores; core c -> (batch b = c//2, head-group g = c%2 of 8 heads).
Data-parallel on B, tensor-parallel on H.  Each core computes a full-shape
[C, N] (transposed) partial of the output projection for its head slice; the
host transposes, sums the two partials per batch and adds proj_b.

v2 layout (matmuls bf16, fp32 PSUM):
  p1 QKV: DMA-prioritized (wk+xp first so kT matmuls start ~6us in), then
     wv (v), then xf+wq (qT).  kT/qT computed transposed [c_out, token],
     v natural [token, c_out] with a ones column per head for softmax sums.
  p2 attention, per (q-block 1024, head-pair): the two heads of a pair live
     in disjoint partition halves of kTt/qTt, so their ST weight loads hit
     alternating PE row groups and pull ahead of in-flight matmuls.
       ST[128k,1024q] = kT^T.T @ qT   (2 matmuls / LDW)
       P = exp(ST) * exp_biasT        (ACT exp; DVE/GPSIMD split multiply)
       pv += [v|1].T @ P              (accumulated over key chunks)
     Normalize: pv evacuated PSUM->SBUF f32 (frees the PSUM bank fast),
     rowsum row spread-DMA'd across partitions for a parallel reciprocal,
     DRAM-bounce stride-0 broadcast, final multiply on GPSIMD.
  p3 proj transposed with weight chunks reused across 4 query blocks.

Mask compaction: keys permuted per batch so unmasked keys come first; only
the first KU (= roundup128(max unmasked count)) keys kept.  Dropped keys are
masked and contribute exactly 0 in the reference too.
"""
import os
import sys

sys.path.insert(0, "/opt/trn_rl_repo")

import numpy as np
import ml_dtypes
from contextlib import ExitStack

import concourse.bass as bass
import concourse.bacc as bacc
import concourse.tile as tile
from concourse import mybir
from concourse.bass_utils import run_bass_kernel_spmd

F32 = mybir.dt.float32
F32R = mybir.dt.float32r
BF16 = mybir.dt.bfloat16
AF = mybir.ActivationFunctionType
NPBF = ml_dtypes.bfloat16

B, N, C, H, D = 4, 2048, 1024, 16, 64
HG = 8            # heads per core
CG = HG * D       # 512: per-core c_out slice of q/k/v and of proj input
P = 128
E = D + 2         # 66: v columns + ones column + pad (4B-aligned bf16 slices)
MASK_VALUE = -65504.0
SCALE = float(D) ** -0.5

_prog_cache = {}


def _ceil_div(a, b):
    return (a + b - 1) // b


def _build(KU):
    """Build the SPMD Bass program (same on all 8 cores) for KU kept keys."""
    KC = KU // P               # number of 128-token key chunks
    QB = N // 512              # 4 query blocks of 512

    nc = bacc.Bacc("TRN2", target_bir_lowering=False, debug=False, num_devices=8)
    xT_d = nc.declare_dram_parameter("xT", [C, N], BF16, isOutput=False)
    xpT_d = nc.declare_dram_parameter("xpT", [C, KU], BF16, isOutput=False)
    expb_d = nc.declare_dram_parameter("expbT", [KU, N], BF16, isOutput=False)
    wq_d = nc.declare_dram_parameter("wq", [P, 8 * CG], BF16, isOutput=False)
    wk_d = nc.declare_dram_parameter("wk", [P, 8 * CG], BF16, isOutput=False)
    wv_d = nc.declare_dram_parameter("wv", [P, 8 * CG], BF16, isOutput=False)
    wp_d = nc.declare_dram_parameter("wp", [P, 4 * C], BF16, isOutput=False)
    qb_d = nc.declare_dram_parameter("qb", [CG], F32, isOutput=False)
    vb_d = nc.declare_dram_parameter("vb", [1, CG], F32, isOutput=False)
    ones_d = nc.declare_dram_parameter("ones", [1, P], F32, isOutput=False)
    vones_d = nc.declare_dram_parameter("vones", [P, HG * E], BF16, isOutput=False)
    outp_d = nc.declare_dram_parameter("outp", [C, N], F32, isOutput=True)

    scr_d = nc.dram_tensor("rs_scratch", [16, 1024], F32)

    with ExitStack() as ctx:
        tc = ctx.enter_context(tile.TileContext(nc))
        persist = ctx.enter_context(tc.tile_pool(name="persist", bufs=1))
        const = ctx.enter_context(tc.tile_pool(name="const", bufs=1))

        ones1 = const.tile([1, P], F32R, name="ones1")
        nc.sync.dma_start(ones1[:], ones_d[:].bitcast(F32R))
        vb_t = const.tile([1, CG], F32R, name="vb_t")
        nc.sync.dma_start(vb_t[:], vb_d[:].bitcast(F32R))
        qb_t = const.tile([P, 4], F32, name="qb_t")
        for m in range(4):
            nc.sync.dma_start(
                qb_t[:, m : m + 1],
                qb_d[m * P : (m + 1) * P].rearrange("(p o) -> p o", o=1),
            )

        qTt = [persist.tile([P, N], BF16, name=f"qT{i}") for i in range(4)]
        kTt = [persist.tile([P, KU], BF16, name=f"kT{i}") for i in range(4)]
        vat = [persist.tile([P, HG * E], BF16, name=f"va{i}") for i in range(KC)]
        ott = [persist.tile([P, N], BF16, name=f"ot{i}") for i in range(4)]
        wp_t = persist.tile([P, 4 * C], BF16, name="wp_t")

        # ---------------- Phase 1: QKV ----------------
        # DMA issue order = compute priority: kT (wk+xp) first, then v (wv),
        # then qT (xf+wq).  Weights/x stay resident so each weight chunk is
        # loaded once.
        with nc.named_scope("p1_qkv"), tc.tile_pool(
            name="wqkv", bufs=1
        ) as wpool, tc.tile_pool(name="xres", bufs=1) as xres, tc.tile_pool(
            name="psq", bufs=4, space="PSUM"
        ) as psq:
            wq_t = wpool.tile([P, 8 * CG], BF16, name="wq_t")
            wk_t = wpool.tile([P, 8 * CG], BF16, name="wk_t")
            wv_t = wpool.tile([P, 8 * CG], BF16, name="wv_t")
            xf = [xres.tile([P, N], BF16, name=f"xf{k}") for k in range(8)]
            xp = [xres.tile([P, KU], BF16, name=f"xp{k}") for k in range(8)]

            # priority order: wk, xp -> wv, vones -> xf, wq
            for _j in range(8):
                nc.sync.dma_start(
                    wk_t[:, _j * CG : (_j + 1) * CG],
                    wk_d[:, _j * CG : (_j + 1) * CG],
                )
            for k in range(8):
                nc.sync.dma_start(xp[k][:], xpT_d[k * P : (k + 1) * P, :])
            for _j in range(8):
                nc.sync.dma_start(
                    wv_t[:, _j * CG : (_j + 1) * CG],
                    wv_d[:, _j * CG : (_j + 1) * CG],
                )
            for tm in range(KC):
                nc.sync.dma_start(vat[tm][:], vones_d[:])
            for k in range(8):
                nc.sync.dma_start(xf[k][:], xT_d[k * P : (k + 1) * P, :])
            for _j in range(8):
                nc.sync.dma_start(
                    wq_t[:, _j * CG : (_j + 1) * CG],
                    wq_d[:, _j * CG : (_j + 1) * CG],
                )

            # kT [c_out, token] over KU: weight chunk reused across 3 blocks
            kblks = [(b0, min(512, KU - b0)) for b0 in range(0, KU, 512)]
            for m in range(4):
                pss = [
                    psq.tile([P, 512], F32, name="ps_k", tag="ps")
                    for _ in range(len(kblks))
                ]
                for kc8 in range(8):
                    lw = wk_t[:, kc8 * CG + m * P : kc8 * CG + (m + 1) * P]
                    for i, (b0, w) in enumerate(kblks):
                        nc.tensor.matmul(
                            pss[i][:, :w],
                            lhsT=lw,
                            rhs=xp[kc8][:, b0 : b0 + w],
                            start=(kc8 == 0),
                            stop=(kc8 == 7),
                        )
                for i, (b0, w) in enumerate(kblks):
                    nc.scalar.activation(
                        kTt[m][:, b0 : b0 + w], pss[i][:, :w], AF.Copy
                    )

            # v natural [token, c_out] + ones/pad columns
            for tm in range(KC):
                psv = psq.tile([P, CG], F32, name="ps_v", tag="ps")
                for kc8 in range(8):
                    nc.tensor.matmul(
                        psv[:],
                        lhsT=xp[kc8][:, tm * P : (tm + 1) * P],
                        rhs=wv_t[:, kc8 * CG : (kc8 + 1) * CG],
                        start=(kc8 == 0),
                        stop=False,
                    )
                nc.tensor.matmul(
                    psv[:],
                    lhsT=ones1[0:1, :],
                    rhs=vb_t[0:1, :],
                    start=False,
                    stop=True,
                )
                nc.vector.tensor_copy(
                    vat[tm][:].rearrange("p (h e) -> p h e", e=E)[:, :, 0:D],
                    psv[:].rearrange("p (h e) -> p h e", e=D),
                )

            # qT [c_out, token] over all N: weight chunk reused across 4 blocks
            for m in range(4):
                pss = [
                    psq.tile([P, 512], F32, name="ps_q", tag="ps") for _ in range(QB)
                ]
                for kc8 in range(8):
                    lw = wq_t[:, kc8 * CG + m * P : kc8 * CG + (m + 1) * P]
                    for nb in range(QB):
                        nc.tensor.matmul(
                            pss[nb][:],
                            lhsT=lw,
                            rhs=xf[kc8][:, nb * 512 : (nb + 1) * 512],
                            start=(kc8 == 0),
                            stop=(kc8 == 7),
                        )
                for nb in range(QB):
                    nc.scalar.activation(
                        qTt[m][:, nb * 512 : (nb + 1) * 512],
                        pss[nb][:],
                        AF.Identity,
                        bias=qb_t[:, m : m + 1],
                    )

        # ---------------- Phase 2: attention (interleaved head pairs) -----
        nc.sync.dma_start(wp_t[:], wp_d[:])  # p3 weights, loads in background
        with nc.named_scope("p2_attn"), tc.tile_pool(
            name="bsb", bufs=KC + 3
        ) as bpool, tc.tile_pool(name="pp", bufs=4) as ppool, tc.tile_pool(
            name="ovp", bufs=4
        ) as ovpool, tc.tile_pool(name="rsp", bufs=4) as rpool, tc.tile_pool(
            name="bcp", bufs=4
        ) as bcpool, tc.tile_pool(
            name="pst", bufs=2, space="PSUM"
        ) as pst, tc.tile_pool(
            name="ppv", bufs=2, space="PSUM"
        ) as ppv:
            for qp in range(QB // 2):
                q0 = qp * 1024
                btiles = []
                for kc in range(KC):
                    bt = bpool.tile([P, 1024], BF16, name="b_t", tag="bt")
                    nc.sync.dma_start(
                        bt[:], expb_d[kc * P : (kc + 1) * P, q0 : q0 + 1024]
                    )
                    btiles.append(bt)
                for t in range(4):
                    pvs = [
                        ppv.tile([P, 1024], F32, name="pv_t", tag="pv")
                        for _ in range(2)
                    ]
                    for kc in range(KC):
                        for hh in range(2):
                            h = 2 * t + hh
                            po = hh * D
                            stt = pst.tile([P, 1024], F32, name="st_t", tag="stt")
                            lw = kTt[t][po : po + D, kc * P : (kc + 1) * P]
                            for j in range(2):
                                nc.tensor.matmul(
                                    stt[:, j * 512 : (j + 1) * 512],
                                    lhsT=lw,
                                    rhs=qTt[t][
                                        po : po + D,
                                        q0 + j * 512 : q0 + (j + 1) * 512,
                                    ],
                                    start=True,
                                    stop=True,
                                )
                            pt = ppool.tile([P, 1024], BF16, name="p_t", tag="pt")
                            nc.scalar.activation(pt[:], stt[:], AF.Exp)
                            nc.vector.tensor_mul(pt[:], pt[:], btiles[kc][:])
                            lv = vat[kc][:, h * E : (h + 1) * E]
                            for j in range(2):
                                nc.tensor.matmul(
                                    pvs[hh][0:E, j * 512 : (j + 1) * 512],
                                    lhsT=lv,
                                    rhs=pt[:, j * 512 : (j + 1) * 512],
                                    start=(kc == 0),
                                    stop=(kc == KC - 1),
                                )
                    for hh in range(2):
                        h = 2 * t + hh
                        po = hh * D
                        it = qp * HG + h
                        # evacuate pv so the PSUM bank frees fast; normalize
                        # from the SBUF copy.
                        ov = ovpool.tile([P, 1024], F32, name="ov_t", tag="ov")
                        nc.vector.tensor_copy(ov[0:E, :], pvs[hh][0:E, :])
                        rsw = rpool.tile([P, 8], F32, name="rsw_t", tag="rsw")
                        nc.sync.dma_start(rsw[:, :], ov[D : D + 1, :])
                        rsw2 = rpool.tile([P, 8], F32, name="rsw2_t", tag="rsw2")
                        nc.vector.reciprocal(rsw2[:, :], rsw[:, :])
                        nc.sync.dma_start(scr_d[it : it + 1, :], rsw2[:, :])
                        bcs = bcpool.tile([D, 1024], F32, name="bcs_t", tag="bcs")
                        row = scr_d[it : it + 1, :]
                        nc.gpsimd.dma_start(
                            bcs[:, :],
                            bass.AP(
                                tensor=row.tensor,
                                offset=row.offset,
                                ap=[[0, D], [1, 1024]],
                            ),
                        )
                        nc.gpsimd.tensor_mul(
                            ott[t][po : po + D, q0 : q0 + 1024],
                            ov[0:D, :],
                            bcs[:, :],
                        )

        # ---------------- Phase 3: projection (transposed output) ---------
        with nc.named_scope("p3_proj"), tc.tile_pool(
            name="oev", bufs=4
        ) as oev, tc.tile_pool(name="psp", bufs=4, space="PSUM") as psp:
            for cm in range(C // P):
                pss = [
                    psp.tile([P, 512], F32, name="ps_p", tag="psp") for _ in range(QB)
                ]
                for t in range(4):
                    lw = wp_t[:, t * C + cm * P : t * C + (cm + 1) * P]
                    for qs in range(QB):
                        nc.tensor.matmul(
                            pss[qs][:],
                            lhsT=lw,
                            rhs=ott[t][:, qs * 512 : (qs + 1) * 512],
                            start=(t == 0),
                            stop=(t == 3),
                        )
                for qs in range(QB):
                    osb = oev.tile([P, 512], F32, name="o_sb", tag="osb")
                    nc.scalar.activation(osb[:], pss[qs][:], AF.Copy)
                    nc.sync.dma_start(
                        outp_d[cm * P : (cm + 1) * P, qs * 512 : (qs + 1) * 512],
                        osb[:],
                    )
    nc.finalize()
    return nc


def kernel(
    x=None,
    attention_mask=None,
    attention_bias=None,
    qkv_w=None,
    q_bias=None,
    v_bias=None,
    proj_w=None,
    proj_b=None,
):
    x = np.ascontiguousarray(np.asarray(x, dtype=np.float32))
    mask = np.asarray(attention_mask).astype(bool)
    bias = np.asarray(attention_bias, dtype=np.float32)
    qkv_w = np.asarray(qkv_w, dtype=np.float32)
    q_bias = np.asarray(q_bias, dtype=np.float32)
    v_bias = np.asarray(v_bias, dtype=np.float32)
    proj_w = np.asarray(proj_w, dtype=np.float32)
    proj_b = np.asarray(proj_b, dtype=np.float32)

    assert x.shape == (B, N, C), x.shape

    # --- mask compaction: unmasked keys first, keep KU of them ---
    perms, us = [], []
    for b in range(B):
        perms.append(np.argsort(mask[b], kind="stable"))
        us.append(int((~mask[b]).sum()))
    KU = min(N, max(P, _ceil_div(max(us), P) * P))

    if KU not in _prog_cache:
        _prog_cache[KU] = _build(KU)
    nc = _prog_cache[KU]

    ones_h = np.ones((1, P), dtype=np.float32)
    vones_h = np.zeros((P, HG * E), dtype=NPBF)
    vones_h.reshape(P, HG, E)[:, :, D] = 1.0
    mv = np.float32(MASK_VALUE)

    per_b = []
    for b in range(B):
        perm = perms[b][:KU]
        xT = np.ascontiguousarray(x[b].T.astype(NPBF))
        xpT = np.ascontiguousarray(x[b][perm].T.astype(NPBF))
        biasT = bias[b].T[perm] + np.where(mask[b][perm], mv, np.float32(0.0))[:, None]
        expbT = np.ascontiguousarray(np.exp(biasT, dtype=np.float32).astype(NPBF))
        per_b.append((xT, xpT, expbT))

    per_g = []
    for g in range(2):
        sl = slice(g * CG, (g + 1) * CG)

        def tile_w(wT, ncols):  # [C_in, ncols] -> [128, (C_in//128)*ncols]
            return np.ascontiguousarray(
                wT.reshape(wT.shape[0] // P, P, ncols)
                .transpose(1, 0, 2)
                .reshape(P, -1)
                .astype(NPBF)
            )

        wq = tile_w((qkv_w[sl, :] * np.float32(SCALE)).T.astype(np.float32), CG)
        wk = tile_w(np.ascontiguousarray(qkv_w[C + g * CG : C + (g + 1) * CG, :].T), CG)
        wv = tile_w(
            np.ascontiguousarray(qkv_w[2 * C + g * CG : 2 * C + (g + 1) * CG, :].T), CG
        )
        wp = tile_w(np.ascontiguousarray(proj_w[:, sl].T), C)
        qb = np.ascontiguousarray(q_bias[sl] * np.float32(SCALE))
        vb = np.ascontiguousarray(v_bias[sl][None, :])
        per_g.append((wq, wk, wv, wp, qb, vb))

    in_maps = []
    for c in range(8):
        b, g = c // 2, c % 2
        xT, xpT, expbT = per_b[b]
        wq, wk, wv, wp, qb, vb = per_g[g]
        in_maps.append(
            {
                "xT": xT,
                "xpT": xpT,
                "expbT": expbT,
                "wq": wq,
                "wk": wk,
                "wv": wv,
                "wp": wp,
                "qb": qb,
                "vb": vb,
                "ones": ones_h,
                "vones": vones_h,
            }
        )

    trace = bool(int(os.environ.get("KBENCH_TRACE", "0")))
    kw = {}
    if trace:
        kw = dict(
            trace=True,
            trace_cores=[
                int(t) for t in os.environ.get("KBENCH_TRACE_CORES", "0").split(",")
            ],
        )
    res = run_bass_kernel_spmd(nc, in_maps, list(range(8)), **kw)
    if trace:
        kernel.last_exec_ns = res.exec_time_ns
        kernel.last_result = res

    out = np.empty((B, N, C), dtype=np.float32)
    for b in range(B):
        outT = res.results[2 * b]["outp"] + res.results[2 * b + 1]["outp"]
        out[b] = outT.T
        out[b] += proj_b[None, :]
    return out


kernel.last_exec_ns = None
kernel.last_result = None


# revision 4
# speedup vs baseline: 1.4330x; 1.4330x over previous
"""Trainium2 Bass kernel for nn_Attention (B=4, N=2048, C=1024, H=16).

Sharding: 8 cores; core c -> (batch b = c//2, head-group g = c%2 of 8 heads).
Data-parallel on B, tensor-parallel on H.  Each core computes a full-shape
[C, N] (transposed) partial of the output projection for its head slice; the
host transposes, sums the two partials per batch and adds proj_b.

v4: the PE clock gate (HAM) throttles to 1.2 GHz whenever the PE idles for a
~3.4us window, and an ACT-bound attention phase leaves exactly such idle --
the whole phase then runs at half clock (measured: K=4/8 for 293us).  So the
kernel keeps the PE dense end-to-end by software-pipelining the dense GEMMs
into the attention loop:

  p1: DMA-prioritized loads; v (all key chunks), kT/qT for head-pair 0 only.
  p2: per (q-block 1024, head-pair, head): ST scores -> ACT exp -> DVE mul by
      exp(bias) -> PV accumulate.  PSUM: ST 2x[128,1024] + PV 1x[128,1024]
      + 2 banks for fused filler matmuls (kT/qT of the next pair during the
      qp=0 pass, the qp=0 projection during the qp=1 pass), which keep the
      PE busy through the phase.  pv is evacuated to SBUF right after its
      last accumulate (frees the bank); the softmax row-sum row is spread
      across partitions by DMA for a parallel reciprocal, broadcast via a
      DRAM-bounce stride-0 DMA, and the final normalize multiply runs on the
      otherwise-idle GPSIMD engine.
  p3: projection for the qp=1 half + output drain.

Mask compaction: keys permuted per batch so unmasked keys come first; only
the first KU (= roundup128(max unmasked count)) keys kept.  Dropped keys are
masked and contribute exactly 0 in the reference too.
"""
import os
import sys

sys.path.insert(0, "/opt/trn_rl_repo")

import numpy as np
import ml_dtypes
from contextlib import ExitStack

import concourse.bass as bass
import concourse.bacc as bacc
import concourse.tile as tile
from concourse import mybir
from concourse.bass_utils import run_bass_kernel_spmd

F32 = mybir.dt.float32
F32R = mybir.dt.float32r
BF16 = mybir.dt.bfloat16
AF = mybir.ActivationFunctionType
NPBF = ml_dtypes.bfloat16

B, N, C, H, D = 4, 2048, 1024, 16, 64
HG = 8            # heads per core
CG = HG * D       # 512: per-core c_out slice of q/k/v and of proj input
P = 128
E = D + 2         # 66: v columns + ones column + pad (4B-aligned bf16 slices)
MASK_VALUE = -65504.0
SCALE = float(D) ** -0.5

_prog_cache = {}


def _ceil_div(a, b):
    return (a + b - 1) // b


def _build(KU, use_qb):
    """Build the SPMD Bass program (same on all 8 cores) for KU kept keys."""
    KC = KU // P               # number of 128-token key chunks
    QB = N // 512              # 4 query blocks of 512

    nc = bacc.Bacc("TRN2", target_bir_lowering=False, debug=False, num_devices=8)
    xT_d = nc.declare_dram_parameter("xT", [C, N], BF16, isOutput=False)
    xpT_d = nc.declare_dram_parameter("xpT", [C, KU], BF16, isOutput=False)
    expb_d = nc.declare_dram_parameter("expbT", [KU, N], BF16, isOutput=False)
    wq_d = nc.declare_dram_parameter("wq", [P, 8 * CG], BF16, isOutput=False)
    wk_d = nc.declare_dram_parameter("wk", [P, 8 * CG], BF16, isOutput=False)
    wv_d = nc.declare_dram_parameter("wv", [P, 8 * CG], BF16, isOutput=False)
    wp_d = nc.declare_dram_parameter("wp", [P, 4 * C], BF16, isOutput=False)
    qb_d = nc.declare_dram_parameter("qb", [CG], F32, isOutput=False)
    vb_d = nc.declare_dram_parameter("vb", [1, CG], F32, isOutput=False)
    ones_d = nc.declare_dram_parameter("ones", [1, P], F32, isOutput=False)
    vones_d = nc.declare_dram_parameter("vones", [P, HG * E], BF16, isOutput=False)
    outp_d = nc.declare_dram_parameter("outp", [C, N], F32, isOutput=True)

    scr_d = nc.dram_tensor("rs_scratch", [16, 1024], F32)

    with ExitStack() as ctx:
        tc = ctx.enter_context(tile.TileContext(nc))
        persist = ctx.enter_context(tc.tile_pool(name="persist", bufs=1))
        const = ctx.enter_context(tc.tile_pool(name="const", bufs=1))

        ones1 = const.tile([1, P], F32R, name="ones1")
        nc.sync.dma_start(ones1[:], ones_d[:].bitcast(F32R))
        vb_t = const.tile([1, CG], F32R, name="vb_t")
        nc.sync.dma_start(vb_t[:], vb_d[:].bitcast(F32R))
        qb_t = const.tile([P, 4], F32, name="qb_t")
        for m in range(4):
            nc.sync.dma_start(
                qb_t[:, m : m + 1],
                qb_d[m * P : (m + 1) * P].rearrange("(p o) -> p o", o=1),
            )
        vo_t = const.tile([P, HG * E], BF16, name="vo_t")

        qTt = [persist.tile([P, N], BF16, name=f"qT{i}") for i in range(4)]
        kTt = [persist.tile([P, KU], BF16, name=f"kT{i}") for i in range(4)]
        vat = [persist.tile([P, HG * E], BF16, name=f"va{i}") for i in range(KC)]
        ott = [persist.tile([P, N], BF16, name=f"ot{i}") for i in range(4)]
        wp_t = persist.tile([P, 4 * C], BF16, name="wp_t")
        # x and qkv weights stay resident through p2 (fused QKV filler)
        wq_t = persist.tile([P, 8 * CG], BF16, name="wq_t")
        wk_t = persist.tile([P, 8 * CG], BF16, name="wk_t")
        wv_t = persist.tile([P, 8 * CG], BF16, name="wv_t")
        xf = [persist.tile([P, N], BF16, name=f"xf{k}") for k in range(8)]
        xp = [persist.tile([P, KU], BF16, name=f"xp{k}") for k in range(8)]

        kblks = [(b0, min(512, KU - b0)) for b0 in range(0, KU, 512)]

        # ---- DMA issue order = compute priority ----
        for _j in range(8):
            nc.sync.dma_start(
                wk_t[:, _j * CG : (_j + 1) * CG], wk_d[:, _j * CG : (_j + 1) * CG]
            )
        half = KU // 2
        for k in range(8):
            nc.sync.dma_start(xp[k][:, 0:half], xpT_d[k * P : (k + 1) * P, 0:half])
            nc.sync.dma_start(
                xp[k][:, half:KU], xpT_d[k * P : (k + 1) * P, half:KU]
            )
        for _j in range(8):
            nc.sync.dma_start(
                wv_t[:, _j * CG : (_j + 1) * CG], wv_d[:, _j * CG : (_j + 1) * CG]
            )
        nc.sync.dma_start(vo_t[:], vones_d[:])
        for k in range(8):
            nc.sync.dma_start(xf[k][:, 0:1024], xT_d[k * P : (k + 1) * P, 0:1024])
            nc.sync.dma_start(xf[k][:, 1024:N], xT_d[k * P : (k + 1) * P, 1024:N])
        for _j in range(8):
            nc.sync.dma_start(
                wq_t[:, _j * CG : (_j + 1) * CG], wq_d[:, _j * CG : (_j + 1) * CG]
            )
        for _j in range(8):
            nc.sync.dma_start(
                wp_t[:, _j * 512 : (_j + 1) * 512], wp_d[:, _j * 512 : (_j + 1) * 512]
            )

        # ---- emit helpers (used in p1 and as p2 filler) ----
        def emit_kT_block(m, i, psq):
            b0, w = kblks[i]
            ps = psq.tile([P, 512], F32, name="ps_k", tag="fps")
            for kc8 in range(8):
                lw = wk_t[:, kc8 * CG + m * P : kc8 * CG + (m + 1) * P]
                nc.tensor.matmul(
                    ps[:, :w],
                    lhsT=lw,
                    rhs=xp[kc8][:, b0 : b0 + w],
                    start=(kc8 == 0),
                    stop=(kc8 == 7),
                )
            return ps, b0, w

        def emit_qT_block(m, nb, psq):
            ps = psq.tile([P, 512], F32, name="ps_q", tag="fps")
            for kc8 in range(8):
                lw = wq_t[:, kc8 * CG + m * P : kc8 * CG + (m + 1) * P]
                nc.tensor.matmul(
                    ps[:],
                    lhsT=lw,
                    rhs=xf[kc8][:, nb * 512 : (nb + 1) * 512],
                    start=(kc8 == 0),
                    stop=(kc8 == 7),
                )
            return ps

        def emit_v_chunk(tm, psq):
            psv = psq.tile([P, CG], F32, name="ps_v", tag="fps")
            for kc8 in range(8):
                nc.tensor.matmul(
                    psv[:],
                    lhsT=xp[kc8][:, tm * P : (tm + 1) * P],
                    rhs=wv_t[:, kc8 * CG : (kc8 + 1) * CG],
                    start=(kc8 == 0),
                    stop=False,
                )
            nc.tensor.matmul(
                psv[:], lhsT=ones1[0:1, :], rhs=vb_t[0:1, :], start=False, stop=True
            )
            nc.vector.tensor_copy(vat[tm][:], vo_t[:])
            nc.vector.tensor_copy(
                vat[tm][:].rearrange("p (h e) -> p h e", e=E)[:, :, 0:D],
                psv[:].rearrange("p (h e) -> p h e", e=D),
            )

        def emit_proj_cq(cm, qs, psq):
            ps = psq.tile([P, 512], F32, name="ps_p", tag="fps")
            for t4 in range(4):
                lw = wp_t[:, t4 * C + cm * P : t4 * C + (cm + 1) * P]
                nc.tensor.matmul(
                    ps[:],
                    lhsT=lw,
                    rhs=ott[t4][:, qs * 512 : (qs + 1) * 512],
                    start=(t4 == 0),
                    stop=(t4 == 3),
                )
            osb = persistless_oev.tile([P, 512], F32, name="o_sb", tag="osb")
            nc.vector.tensor_copy(osb[:], ps[:])
            nc.sync.dma_start(
                outp_d[cm * P : (cm + 1) * P, qs * 512 : (qs + 1) * 512], osb[:]
            )

        # ---------------- Phase 1 ----------------
        with nc.named_scope("p1_qkv"), tc.tile_pool(
            name="psq1", bufs=4, space="PSUM"
        ) as psq1:
            # kT pair 0
            for i in range(len(kblks)):
                ps, b0, w = emit_kT_block(0, i, psq1)
                nc.scalar.activation(kTt[0][:, b0 : b0 + w], ps[:, :w], AF.Copy)
            # v all chunks
            for tm in range(KC):
                emit_v_chunk(tm, psq1)
            # qT pair 0
            for nb in range(QB):
                ps = emit_qT_block(0, nb, psq1)
                if use_qb:
                    nc.scalar.activation(
                        qTt[0][:, nb * 512 : (nb + 1) * 512],
                        ps[:],
                        AF.Identity,
                        bias=qb_t[:, 0:1],
                    )
                else:
                    nc.scalar.activation(
                        qTt[0][:, nb * 512 : (nb + 1) * 512], ps[:], AF.Copy
                    )

        # ---------------- Phase 2: attention + fused filler ---------------
        with nc.named_scope("p2_attn"), tc.tile_pool(
            name="bsb", bufs=KC + 1
        ) as bpool, tc.tile_pool(name="pp", bufs=4) as ppool, tc.tile_pool(
            name="ovp", bufs=3
        ) as ovpool, tc.tile_pool(name="rsp", bufs=4) as rpool, tc.tile_pool(
            name="oev2", bufs=3
        ) as oev2, tc.tile_pool(name="bcp", bufs=2) as bcpool, tc.tile_pool(
            name="pst", bufs=2, space="PSUM"
        ) as pst, tc.tile_pool(
            name="ppv", bufs=1, space="PSUM"
        ) as ppv, tc.tile_pool(
            name="fps", bufs=2, space="PSUM"
        ) as fps:
            persistless_oev = oev2

            def filler_for(qp, t):
                """list of thunks to interleave into slot (qp, t)."""
                th = []
                if qp == 0 and t < 3:
                    m = t + 1
                    for i in range(len(kblks)):
                        def _k(i=i, m=m):
                            ps, b0, w = emit_kT_block(m, i, fps)
                            nc.vector.tensor_copy(
                                kTt[m][:, b0 : b0 + w], ps[:, :w]
                            )
                        th.append(_k)
                    for nb in range(QB):
                        def _q(nb=nb, m=m):
                            ps = emit_qT_block(m, nb, fps)
                            if use_qb:
                                nc.scalar.activation(
                                    qTt[m][:, nb * 512 : (nb + 1) * 512],
                                    ps[:],
                                    AF.Identity,
                                    bias=qb_t[:, m : m + 1],
                                )
                            else:
                                nc.vector.tensor_copy(
                                    qTt[m][:, nb * 512 : (nb + 1) * 512], ps[:]
                                )
                        th.append(_q)
                if qp == 1 and t in (0, 1):
                    for cm in range(t * 4, t * 4 + 4):
                        for qs in range(2):
                            th.append(lambda cm=cm, qs=qs: emit_proj_cq(cm, qs, fps))
                return th

            for qp in range(QB // 2):
                q0 = qp * 1024
                btiles = []
                for kc in range(KC):
                    bt = bpool.tile([P, 1024], BF16, name="b_t", tag="bt")
                    nc.sync.dma_start(
                        bt[:], expb_d[kc * P : (kc + 1) * P, q0 : q0 + 1024]
                    )
                    btiles.append(bt)
                for t in range(4):
                    th = filler_for(qp, t)
                    # spread filler across the 2*KC inner steps
                    nsteps = 2 * KC
                    sched = {}
                    for i, fn in enumerate(th):
                        step = min(nsteps - 1, (i * nsteps) // max(len(th), 1) + 1)
                        sched.setdefault(step, []).append(fn)
                    step = 0
                    for hh in range(2):
                        h = 2 * t + hh
                        po = hh * D
                        pv = ppv.tile([P, 1024], F32, name="pv_t", tag="pv")
                        for kc in range(KC):
                            stt = pst.tile([P, 1024], F32, name="st_t", tag="stt")
                            lw = kTt[t][po : po + D, kc * P : (kc + 1) * P]
                            for j in range(2):
                                nc.tensor.matmul(
                                    stt[:, j * 512 : (j + 1) * 512],
                                    lhsT=lw,
                                    rhs=qTt[t][
                                        po : po + D,
                                        q0 + j * 512 : q0 + (j + 1) * 512,
                                    ],
                                    start=True,
                                    stop=True,
                                )
                            pt = ppool.tile([P, 1024], BF16, name="p_t", tag="pt")
                            nc.scalar.activation(pt[:], stt[:], AF.Exp)
                            nc.vector.tensor_mul(pt[:], pt[:], btiles[kc][:])
                            lv = vat[kc][:, h * E : (h + 1) * E]
                            for j in range(2):
                                nc.tensor.matmul(
                                    pv[0:E, j * 512 : (j + 1) * 512],
                                    lhsT=lv,
                                    rhs=pt[:, j * 512 : (j + 1) * 512],
                                    start=(kc == 0),
                                    stop=(kc == KC - 1),
                                )
                            for fn in sched.get(step, []):
                                fn()
                            step += 1
                        # evacuate pv fast, normalize from the SBUF copy
                        it = qp * HG + h
                        ov = ovpool.tile([P, 1024], F32, name="ov_t", tag="ov")
                        nc.vector.tensor_copy(ov[0:E, :], pv[0:E, :])
                        rsw = rpool.tile([P, 8], F32, name="rsw_t", tag="rsw")
                        nc.sync.dma_start(rsw[:, :], ov[D : D + 1, :])
                        rsw2 = rpool.tile([P, 8], F32, name="rsw2_t", tag="rsw2")
                        nc.vector.reciprocal(rsw2[:, :], rsw[:, :])
                        nc.sync.dma_start(scr_d[it : it + 1, :], rsw2[:, :])
                        bcs = bcpool.tile([D, 1024], F32, name="bcs_t", tag="bcs")
                        row = scr_d[it : it + 1, :]
                        nc.gpsimd.dma_start(
                            bcs[:, :],
                            bass.AP(
                                tensor=row.tensor,
                                offset=row.offset,
                                ap=[[0, D], [1, 1024]],
                            ),
                        )
                        nc.gpsimd.tensor_mul(
                            ott[t][po : po + D, q0 : q0 + 1024],
                            ov[0:D, :],
                            bcs[:, :],
                        )

        # ---------------- Phase 3: projection qp=1 half -------------------
        with nc.named_scope("p3_proj"), tc.tile_pool(
            name="oev", bufs=4
        ) as oev, tc.tile_pool(name="psp", bufs=4, space="PSUM") as psp:
            for cm in range(C // P):
                for qs in range(2, 4):
                    ps = psp.tile([P, 512], F32, name="ps_p3", tag="psp")
                    for t4 in range(4):
                        lw = wp_t[:, t4 * C + cm * P : t4 * C + (cm + 1) * P]
                        nc.tensor.matmul(
                            ps[:],
                            lhsT=lw,
                            rhs=ott[t4][:, qs * 512 : (qs + 1) * 512],
                            start=(t4 == 0),
                            stop=(t4 == 3),
                        )
                    osb = oev.tile([P, 512], F32, name="o_sb3", tag="osb3")
                    nc.scalar.activation(osb[:], ps[:], AF.Copy)
                    nc.sync.dma_start(
                        outp_d[cm * P : (cm + 1) * P, qs * 512 : (qs + 1) * 512],
                        osb[:],
                    )
    nc.finalize()
    return nc


def kernel(
    x=None,
    attention_mask=None,
    attention_bias=None,
    qkv_w=None,
    q_bias=None,
    v_bias=None,
    proj_w=None,
    proj_b=None,
):
    x = np.ascontiguousarray(np.asarray(x, dtype=np.float32))
    mask = np.asarray(attention_mask).astype(bool)
    bias = np.asarray(attention_bias, dtype=np.float32)
    qkv_w = np.asarray(qkv_w, dtype=np.float32)
    q_bias = np.asarray(q_bias, dtype=np.float32)
    v_bias = np.asarray(v_bias, dtype=np.float32)
    proj_w = np.asarray(proj_w, dtype=np.float32)
    proj_b = np.asarray(proj_b, dtype=np.float32)

    assert x.shape == (B, N, C), x.shape

    # --- mask compaction: unmasked keys first, keep KU of them ---
    perms, us = [], []
    for b in range(B):
        perms.append(np.argsort(mask[b], kind="stable"))
        us.append(int((~mask[b]).sum()))
    KU = min(N, max(P, _ceil_div(max(us), P) * P))
    use_qb = bool(np.any(q_bias))

    key = (KU, use_qb)
    if key not in _prog_cache:
        _prog_cache[key] = _build(KU, use_qb)
    nc = _prog_cache[key]

    ones_h = np.ones((1, P), dtype=np.float32)
    vones_h = np.zeros((P, HG * E), dtype=NPBF)
    vones_h.reshape(P, HG, E)[:, :, D] = 1.0
    mv = np.float32(MASK_VALUE)

    per_b = []
    for b in range(B):
        perm = perms[b][:KU]
        xT = np.ascontiguousarray(x[b].T.astype(NPBF))
        xpT = np.ascontiguousarray(x[b][perm].T.astype(NPBF))
        biasT = bias[b].T[perm] + np.where(mask[b][perm], mv, np.float32(0.0))[:, None]
        expbT = np.ascontiguousarray(np.exp(biasT, dtype=np.float32).astype(NPBF))
        per_b.append((xT, xpT, expbT))

    per_g = []
    for g in range(2):
        sl = slice(g * CG, (g + 1) * CG)

        def tile_w(wT, ncols):  # [C_in, ncols] -> [128, (C_in//128)*ncols]
            return np.ascontiguousarray(
                wT.reshape(wT.shape[0] // P, P, ncols)
                .transpose(1, 0, 2)
                .reshape(P, -1)
                .astype(NPBF)
            )

        wq = tile_w((qkv_w[sl, :] * np.float32(SCALE)).T.astype(np.float32), CG)
        wk = tile_w(np.ascontiguousarray(qkv_w[C + g * CG : C + (g + 1) * CG, :].T), CG)
        wv = tile_w(
            np.ascontiguousarray(qkv_w[2 * C + g * CG : 2 * C + (g + 1) * CG, :].T), CG
        )
        wp = tile_w(np.ascontiguousarray(proj_w[:, sl].T), C)
        qb = np.ascontiguousarray(q_bias[sl] * np.float32(SCALE))
        vb = np.ascontiguousarray(v_bias[sl][None, :])
        per_g.append((wq, wk, wv, wp, qb, vb))

    in_maps = []
    for c in range(8):
        b, g = c // 2, c % 2
        xT, xpT, expbT = per_b[b]
        wq, wk, wv, wp, qb, vb = per_g[g]
        in_maps.append(
            {
                "xT": xT,
                "xpT": xpT,
                "expbT": expbT,
                "wq": wq,
                "wk": wk,
                "wv": wv,
                "wp": wp,
                "qb": qb,
                "vb": vb,
                "ones": ones_h,
                "vones": vones_h,
            }
        )

    trace = bool(int(os.environ.get("KBENCH_TRACE", "0")))
    kw = {}
    if trace:
        kw = dict(
            trace=True,
            trace_cores=[
                int(t) for t in os.environ.get("KBENCH_TRACE_CORES", "0").split(",")
            ],
        )
    res = run_bass_kernel_spmd(nc, in_maps, list(range(8)), **kw)
    if trace:
        kernel.last_exec_ns = res.exec_time_ns
        kernel.last_result = res

    out = np.empty((B, N, C), dtype=np.float32)
    for b in range(B):
        outT = res.results[2 * b]["outp"] + res.results[2 * b + 1]["outp"]
        out[b] = outT.T
        out[b] += proj_b[None, :]
    return out


kernel.last_exec_ns = None
kernel.last_result = None


# revision 6
# speedup vs baseline: 1.4932x; 1.0420x over previous
"""Trainium2 Bass kernel for nn_Attention (B=4, N=2048, C=1024, H=16).

Sharding: 8 cores; core c -> (batch b = c//2, head-group g = c%2 of 8 heads).
Data-parallel on B, tensor-parallel on H.  Each core computes a full-shape
[C, N] (transposed) partial of the output projection for its head slice; the
host transposes, sums the two partials per batch and adds proj_b.

v5: the PE clock gate (HAM) throttles to 1.2 GHz whenever the PE has idle
moments across a ~3.4us window; an ACT-bound attention phase then runs at
half clock (measured).  So the kernel oversubscribes the PE end-to-end:

  - One flat scope, no PSUM phase walls: PSUM = ST 2x[128,1024] (4 banks)
    + PV 1x[128,1024] (2 banks) + a 2-bank "filler" pool that carries ALL
    dense-GEMM work (kT/qT/v up front, then next-pair kT/qT and the qp=0
    projection as in-loop filler spread across the attention slots).
  - Attention per (q-block 1024, head): ST scores -> ACT exp -> DVE mul by
    exp(bias) -> PV accumulate; pv evacuated to SBUF immediately (frees the
    bank), row-sum row spread by DMA across partitions for a parallel
    reciprocal, DRAM-bounce stride-0 broadcast, normalize multiply on the
    otherwise-idle GPSIMD.
  - A few dummy matmuls pad slots with no productive filler so the HAM
    activity window never sees idle.

Mask compaction: keys permuted per batch so unmasked keys come first; only
the first KU (= roundup128(max unmasked count)) keys kept.  Dropped keys are
masked and contribute exactly 0 in the reference too.
"""
import os
import sys

sys.path.insert(0, "/opt/trn_rl_repo")

import numpy as np
import ml_dtypes
from contextlib import ExitStack

import concourse.bass as bass
import concourse.bacc as bacc
import concourse.tile as tile
from concourse import mybir
from concourse.bass_utils import run_bass_kernel_spmd

F32 = mybir.dt.float32
F32R = mybir.dt.float32r
BF16 = mybir.dt.bfloat16
AF = mybir.ActivationFunctionType
NPBF = ml_dtypes.bfloat16

B, N, C, H, D = 4, 2048, 1024, 16, 64
HG = 8            # heads per core
CG = HG * D       # 512: per-core c_out slice of q/k/v and of proj input
P = 128
E = D + 2         # 66: v columns + ones column + pad (4B-aligned bf16 slices)
MASK_VALUE = -65504.0
SCALE = float(D) ** -0.5

_prog_cache = {}


def _ceil_div(a, b):
    return (a + b - 1) // b


def _build(KU, use_qb):
    """Build the SPMD Bass program (same on all 8 cores) for KU kept keys."""
    KC = KU // P               # number of 128-token key chunks
    QB = N // 512              # 4 query blocks of 512

    nc = bacc.Bacc("TRN2", target_bir_lowering=False, debug=False, num_devices=8)
    xT_d = nc.declare_dram_parameter("xT", [C, N], BF16, isOutput=False)
    xpT_d = nc.declare_dram_parameter("xpT", [C, KU], BF16, isOutput=False)
    expb_d = nc.declare_dram_parameter("expbT", [KU, N], BF16, isOutput=False)
    wq_d = nc.declare_dram_parameter("wq", [P, 8 * CG], BF16, isOutput=False)
    wk_d = nc.declare_dram_parameter("wk", [P, 8 * CG], BF16, isOutput=False)
    wv_d = nc.declare_dram_parameter("wv", [P, 8 * CG], BF16, isOutput=False)
    wp_d = nc.declare_dram_parameter("wp", [P, 4 * C], BF16, isOutput=False)
    qb_d = nc.declare_dram_parameter("qb", [CG], F32, isOutput=False)
    vb_d = nc.declare_dram_parameter("vb", [1, CG], F32, isOutput=False)
    ones_d = nc.declare_dram_parameter("ones", [1, P], F32, isOutput=False)
    vones_d = nc.declare_dram_parameter("vones", [P, HG * E], BF16, isOutput=False)
    outp_d = nc.declare_dram_parameter("outp", [C, N], F32, isOutput=True)

    scr_d = nc.dram_tensor("rs_scratch", [16, 1024], F32)

    with ExitStack() as ctx:
        tc = ctx.enter_context(tile.TileContext(nc))
        persist = ctx.enter_context(tc.tile_pool(name="persist", bufs=1))
        const = ctx.enter_context(tc.tile_pool(name="const", bufs=1))

        ones1 = const.tile([1, P], F32R, name="ones1")
        nc.sync.dma_start(ones1[:], ones_d[:].bitcast(F32R))
        vb_t = const.tile([1, CG], F32R, name="vb_t")
        nc.sync.dma_start(vb_t[:], vb_d[:].bitcast(F32R))
        qb_t = const.tile([P, 4], F32, name="qb_t")
        for m in range(4):
            nc.sync.dma_start(
                qb_t[:, m : m + 1],
                qb_d[m * P : (m + 1) * P].rearrange("(p o) -> p o", o=1),
            )
        vo_t = const.tile([P, HG * E], BF16, name="vo_t")

        qTt = [persist.tile([P, N], BF16, name=f"qT{i}") for i in range(4)]
        kTt = [persist.tile([P, KU], BF16, name=f"kT{i}") for i in range(4)]
        vat = [persist.tile([P, HG * E], BF16, name=f"va{i}") for i in range(KC)]
        ott = [persist.tile([P, N], BF16, name=f"ot{i}") for i in range(4)]
        wp_t = persist.tile([P, 4 * C], BF16, name="wp_t")
        wq_t = persist.tile([P, 8 * CG], BF16, name="wq_t")
        wk_t = persist.tile([P, 8 * CG], BF16, name="wk_t")
        wv_t = persist.tile([P, 8 * CG], BF16, name="wv_t")
        xf = [persist.tile([P, N], BF16, name=f"xf{k}") for k in range(8)]
        xp = [persist.tile([P, KU], BF16, name=f"xp{k}") for k in range(8)]

        kblks = [(b0, min(512, KU - b0)) for b0 in range(0, KU, 512)]

        # ---- DMA issue order = compute priority ----
        for _j in range(8):
            nc.sync.dma_start(
                wk_t[:, _j * CG : (_j + 1) * CG], wk_d[:, _j * CG : (_j + 1) * CG]
            )
        half = KU // 2
        for k in range(8):
            nc.sync.dma_start(xp[k][:, 0:half], xpT_d[k * P : (k + 1) * P, 0:half])
            nc.sync.dma_start(
                xp[k][:, half:KU], xpT_d[k * P : (k + 1) * P, half:KU]
            )
        for _j in range(8):
            nc.sync.dma_start(
                wq_t[:, _j * CG : (_j + 1) * CG], wq_d[:, _j * CG : (_j + 1) * CG]
            )
        for k in range(8):
            nc.sync.dma_start(xf[k][:, 0:1024], xT_d[k * P : (k + 1) * P, 0:1024])
        for _j in range(8):
            nc.sync.dma_start(
                wv_t[:, _j * CG : (_j + 1) * CG], wv_d[:, _j * CG : (_j + 1) * CG]
            )
        nc.sync.dma_start(vo_t[:], vones_d[:])
        for k in range(8):
            nc.sync.dma_start(xf[k][:, 1024:N], xT_d[k * P : (k + 1) * P, 1024:N])
        for _j in range(8):
            nc.sync.dma_start(
                wp_t[:, _j * 512 : (_j + 1) * 512], wp_d[:, _j * 512 : (_j + 1) * 512]
            )

        with tc.tile_pool(name="bsb", bufs=KC + 2) as bpool, tc.tile_pool(
            name="pp", bufs=4
        ) as ppool, tc.tile_pool(name="ovp", bufs=3) as ovpool, tc.tile_pool(
            name="rsp", bufs=4
        ) as rpool, tc.tile_pool(name="oev2", bufs=3) as oev2, tc.tile_pool(
            name="bcp", bufs=2
        ) as bcpool, tc.tile_pool(
            name="pst", bufs=2, space="PSUM"
        ) as pst, tc.tile_pool(
            name="ppv", bufs=1, space="PSUM"
        ) as ppv, tc.tile_pool(
            name="fps", bufs=2, space="PSUM"
        ) as fps:

            # ---- emit units (each ~1.7-2.1us of PE work through fps) ----
            def emit_kT_block(m, i):
                b0, w = kblks[i]
                ps = fps.tile([P, 512], F32, name="ps_k", tag="fps")
                for kc8 in range(8):
                    lw = wk_t[:, kc8 * CG + m * P : kc8 * CG + (m + 1) * P]
                    nc.tensor.matmul(
                        ps[:, :w],
                        lhsT=lw,
                        rhs=xp[kc8][:, b0 : b0 + w],
                        start=(kc8 == 0),
                        stop=(kc8 == 7),
                    )
                nc.vector.tensor_copy(kTt[m][:, b0 : b0 + w], ps[:, :w])

            def emit_qT_block(m, nb):
                ps = fps.tile([P, 512], F32, name="ps_q", tag="fps")
                for kc8 in range(8):
                    lw = wq_t[:, kc8 * CG + m * P : kc8 * CG + (m + 1) * P]
                    nc.tensor.matmul(
                        ps[:],
                        lhsT=lw,
                        rhs=xf[kc8][:, nb * 512 : (nb + 1) * 512],
                        start=(kc8 == 0),
                        stop=(kc8 == 7),
                    )
                if use_qb:
                    nc.scalar.activation(
                        qTt[m][:, nb * 512 : (nb + 1) * 512],
                        ps[:],
                        AF.Identity,
                        bias=qb_t[:, m : m + 1],
                    )
                else:
                    nc.vector.tensor_copy(
                        qTt[m][:, nb * 512 : (nb + 1) * 512], ps[:]
                    )

            def emit_v_chunk(tm):
                psv = fps.tile([P, CG], F32, name="ps_v", tag="fps")
                for kc8 in range(8):
                    nc.tensor.matmul(
                        psv[:],
                        lhsT=xp[kc8][:, tm * P : (tm + 1) * P],
                        rhs=wv_t[:, kc8 * CG : (kc8 + 1) * CG],
                        start=(kc8 == 0),
                        stop=False,
                    )
                nc.tensor.matmul(
                    psv[:], lhsT=ones1[0:1, :], rhs=vb_t[0:1, :], start=False,
                    stop=True,
                )
                nc.vector.tensor_copy(vat[tm][:], vo_t[:])
                nc.vector.tensor_copy(
                    vat[tm][:].rearrange("p (h e) -> p h e", e=E)[:, :, 0:D],
                    psv[:].rearrange("p (h e) -> p h e", e=D),
                )

            def emit_proj_cq(cm, qs):
                ps = fps.tile([P, 512], F32, name="ps_p", tag="fps")
                for t4 in range(4):
                    lw = wp_t[:, t4 * C + cm * P : t4 * C + (cm + 1) * P]
                    nc.tensor.matmul(
                        ps[:],
                        lhsT=lw,
                        rhs=ott[t4][:, qs * 512 : (qs + 1) * 512],
                        start=(t4 == 0),
                        stop=(t4 == 3),
                    )
                osb = oev2.tile([P, 512], F32, name="o_sb", tag="osb")
                nc.vector.tensor_copy(osb[:], ps[:])
                nc.sync.dma_start(
                    outp_d[cm * P : (cm + 1) * P, qs * 512 : (qs + 1) * 512],
                    osb[:],
                )

            def emit_dummy():
                # keeps the HAM activity window busy; no consumers
                ps = fps.tile([P, 512], F32, name="ps_d", tag="fps")
                nc.tensor.matmul(
                    ps[:], lhsT=wp_t[:, 0:P], rhs=ott[0][:, 0:512],
                    start=True, stop=True,
                )

            # ---- pre-attention GEMMs (kT/qT pair 0 + all v) ----
            for i in range(len(kblks)):
                emit_kT_block(0, i)
            emit_qT_block(0, 0)
            emit_qT_block(0, 1)
            for tm in range(KC):
                emit_v_chunk(tm)

            # ---- filler schedule: slot (qp, t) -> list of thunks ----
            def filler_for(qp, t):
                th = []
                if qp == 0:
                    if t < 3:
                        m = t + 1
                        for i in range(len(kblks)):
                            th.append(lambda m=m, i=i: emit_kT_block(m, i))
                        th.append(lambda m=m: emit_qT_block(m, 0))
                        th.append(lambda m=m: emit_qT_block(m, 1))
                    if t > 0:
                        # qp=1 half of the previous pair's qT, needed at (qp1, t-1)
                        th.append(lambda m=t: emit_qT_block(t, 2))
                        th.append(lambda m=t: emit_qT_block(t, 3))
                    if t == 3:
                        th.append(lambda: emit_qT_block(0, 2))
                        th.append(lambda: emit_qT_block(0, 3))
                        for _ in range(4):
                            th.append(emit_dummy)
                else:
                    for cm in (2 * t, 2 * t + 1):
                        for qs in range(2):
                            th.append(lambda cm=cm, qs=qs: emit_proj_cq(cm, qs))
                    for _ in range(2):
                        th.append(emit_dummy)
                return th

            for qp in range(QB // 2):
                q0 = qp * 1024
                btiles = []
                for kc in range(KC):
                    bt = bpool.tile([P, 1024], BF16, name="b_t", tag="bt")
                    nc.sync.dma_start(
                        bt[:], expb_d[kc * P : (kc + 1) * P, q0 : q0 + 1024]
                    )
                    btiles.append(bt)
                for t in range(4):
                    th = filler_for(qp, t)
                    nsteps = 2 * KC
                    sched = {}
                    for i, fn in enumerate(th):
                        step = min(nsteps - 1, (i * nsteps) // max(len(th), 1) + 1)
                        sched.setdefault(step, []).append(fn)
                    step = 0
                    for hh in range(2):
                        h = 2 * t + hh
                        po = hh * D
                        pv = ppv.tile([P, 1024], F32, name="pv_t", tag="pv")
                        for kc in range(KC):
                            stt = pst.tile([P, 1024], F32, name="st_t", tag="stt")
                            lw = kTt[t][po : po + D, kc * P : (kc + 1) * P]
                            for j in range(2):
                                nc.tensor.matmul(
                                    stt[:, j * 512 : (j + 1) * 512],
                                    lhsT=lw,
                                    rhs=qTt[t][
                                        po : po + D,
                                        q0 + j * 512 : q0 + (j + 1) * 512,
                                    ],
                                    start=True,
                                    stop=True,
                                )
                            pt = ppool.tile([P, 1024], BF16, name="p_t", tag="pt")
                            nc.scalar.activation(pt[:], stt[:], AF.Exp)
                            nc.vector.tensor_mul(pt[:], pt[:], btiles[kc][:])
                            lv = vat[kc][:, h * E : (h + 1) * E]
                            for j in range(2):
                                nc.tensor.matmul(
                                    pv[0:E, j * 512 : (j + 1) * 512],
                                    lhsT=lv,
                                    rhs=pt[:, j * 512 : (j + 1) * 512],
                                    start=(kc == 0),
                                    stop=(kc == KC - 1),
                                )
                            for fn in sched.get(step, []):
                                fn()
                            step += 1
                        # evacuate pv fast, normalize from the SBUF copy
                        it = qp * HG + h
                        ov = ovpool.tile([P, 1024], F32, name="ov_t", tag="ov")
                        nc.vector.tensor_copy(ov[0:E, :], pv[0:E, :])
                        rsw = rpool.tile([P, 8], F32, name="rsw_t", tag="rsw")
                        nc.sync.dma_start(rsw[:, :], ov[D : D + 1, :])
                        rsw2 = rpool.tile([P, 8], F32, name="rsw2_t", tag="rsw2")
                        nc.vector.reciprocal(rsw2[:, :], rsw[:, :])
                        nc.sync.dma_start(scr_d[it : it + 1, :], rsw2[:, :])
                        bcs = bcpool.tile([D, 1024], F32, name="bcs_t", tag="bcs")
                        row = scr_d[it : it + 1, :]
                        nc.gpsimd.dma_start(
                            bcs[:, :],
                            bass.AP(
                                tensor=row.tensor,
                                offset=row.offset,
                                ap=[[0, D], [1, 1024]],
                            ),
                        )
                        nc.gpsimd.tensor_mul(
                            ott[t][po : po + D, q0 : q0 + 1024],
                            ov[0:D, :],
                            bcs[:, :],
                        )

            # ---- projection, qp=1 half ----
            for cm in range(C // P):
                for qs in range(2, 4):
                    emit_proj_cq(cm, qs)
    nc.finalize()
    return nc


def kernel(
    x=None,
    attention_mask=None,
    attention_bias=None,
    qkv_w=None,
    q_bias=None,
    v_bias=None,
    proj_w=None,
    proj_b=None,
):
    x = np.ascontiguousarray(np.asarray(x, dtype=np.float32))
    mask = np.asarray(attention_mask).astype(bool)
    bias = np.asarray(attention_bias, dtype=np.float32)
    qkv_w = np.asarray(qkv_w, dtype=np.float32)
    q_bias = np.asarray(q_bias, dtype=np.float32)
    v_bias = np.asarray(v_bias, dtype=np.float32)
    proj_w = np.asarray(proj_w, dtype=np.float32)
    proj_b = np.asarray(proj_b, dtype=np.float32)

    assert x.shape == (B, N, C), x.shape

    # --- mask compaction: unmasked keys first, keep KU of them ---
    perms, us = [], []
    for b in range(B):
        perms.append(np.argsort(mask[b], kind="stable"))
        us.append(int((~mask[b]).sum()))
    KU = min(N, max(P, _ceil_div(max(us), P) * P))
    use_qb = bool(np.any(q_bias))

    key = (KU, use_qb)
    if key not in _prog_cache:
        _prog_cache[key] = _build(KU, use_qb)
    nc = _prog_cache[key]

    ones_h = np.ones((1, P), dtype=np.float32)
    vones_h = np.zeros((P, HG * E), dtype=NPBF)
    vones_h.reshape(P, HG, E)[:, :, D] = 1.0
    mv = np.float32(MASK_VALUE)

    per_b = []
    for b in range(B):
        perm = perms[b][:KU]
        xT = np.ascontiguousarray(x[b].T.astype(NPBF))
        xpT = np.ascontiguousarray(x[b][perm].T.astype(NPBF))
        biasT = bias[b].T[perm] + np.where(mask[b][perm], mv, np.float32(0.0))[:, None]
        expbT = np.ascontiguousarray(np.exp(biasT, dtype=np.float32).astype(NPBF))
        per_b.append((xT, xpT, expbT))

    per_g = []
    for g in range(2):
        sl = slice(g * CG, (g + 1) * CG)

        def tile_w(wT, ncols):  # [C_in, ncols] -> [128, (C_in//128)*ncols]
            return np.ascontiguousarray(
                wT.reshape(wT.shape[0] // P, P, ncols)
                .transpose(1, 0, 2)
                .reshape(P, -1)
                .astype(NPBF)
            )

        wq = tile_w((qkv_w[sl, :] * np.float32(SCALE)).T.astype(np.float32), CG)
        wk = tile_w(np.ascontiguousarray(qkv_w[C + g * CG : C + (g + 1) * CG, :].T), CG)
        wv = tile_w(
            np.ascontiguousarray(qkv_w[2 * C + g * CG : 2 * C + (g + 1) * CG, :].T), CG
        )
        wp = tile_w(np.ascontiguousarray(proj_w[:, sl].T), C)
        qb = np.ascontiguousarray(q_bias[sl] * np.float32(SCALE))
        vb = np.ascontiguousarray(v_bias[sl][None, :])
        per_g.append((wq, wk, wv, wp, qb, vb))

    in_maps = []
    for c in range(8):
        b, g = c // 2, c % 2
        xT, xpT, expbT = per_b[b]
        wq, wk, wv, wp, qb, vb = per_g[g]
        in_maps.append(
            {
                "xT": xT,
                "xpT": xpT,
                "expbT": expbT,
                "wq": wq,
                "wk": wk,
                "wv": wv,
                "wp": wp,
                "qb": qb,
                "vb": vb,
                "ones": ones_h,
                "vones": vones_h,
            }
        )

    trace = bool(int(os.environ.get("KBENCH_TRACE", "0")))
    kw = {}
    if trace:
        kw = dict(
            trace=True,
            trace_cores=[
                int(t) for t in os.environ.get("KBENCH_TRACE_CORES", "0").split(",")
            ],
        )
    res = run_bass_kernel_spmd(nc, in_maps, list(range(8)), **kw)
    if trace:
        kernel.last_exec_ns = res.exec_time_ns
        kernel.last_result = res

    out = np.empty((B, N, C), dtype=np.float32)
    for b in range(B):
        outT = res.results[2 * b]["outp"] + res.results[2 * b + 1]["outp"]
        out[b] = outT.T
        out[b] += proj_b[None, :]
    return out


kernel.last_exec_ns = None
kernel.last_result = None


# revision 10
# speedup vs baseline: 1.5341x; 1.0274x over previous
"""Trainium2 Bass kernel for nn_Attention (B=4, N=2048, C=1024, H=16).

Sharding: 8 cores; core c -> (batch b = c//2, head-group g = c%2 of 8 heads).
Data-parallel on B, tensor-parallel on H.  Each core computes a full-shape
[C, N] (transposed) partial of the output projection for its head slice; the
host transposes, sums the two partials per batch and adds proj_b.

v5: the PE clock gate (HAM) throttles to 1.2 GHz whenever the PE has idle
moments across a ~3.4us window; an ACT-bound attention phase then runs at
half clock (measured).  So the kernel oversubscribes the PE end-to-end:

  - One flat scope, no PSUM phase walls: PSUM = ST 2x[128,1024] (4 banks)
    + PV 1x[128,1024] (2 banks) + a 2-bank "filler" pool that carries ALL
    dense-GEMM work (kT/qT/v up front, then next-pair kT/qT and the qp=0
    projection as in-loop filler spread across the attention slots).
  - Attention per (q-block 1024, head): ST scores -> ACT exp -> DVE mul by
    exp(bias) -> PV accumulate; pv evacuated to SBUF immediately (frees the
    bank), row-sum row spread by DMA across partitions for a parallel
    reciprocal, DRAM-bounce stride-0 broadcast, normalize multiply on the
    otherwise-idle GPSIMD.
  - A few dummy matmuls pad slots with no productive filler so the HAM
    activity window never sees idle.

Mask compaction: keys permuted per batch so unmasked keys come first; only
the first KU (= roundup128(max unmasked count)) keys kept.  Dropped keys are
masked and contribute exactly 0 in the reference too.
"""
import os
import sys

sys.path.insert(0, "/opt/trn_rl_repo")

import numpy as np
import ml_dtypes
from contextlib import ExitStack

import concourse.bass as bass
import concourse.bacc as bacc
import concourse.tile as tile
from concourse import mybir
from concourse.bass_utils import run_bass_kernel_spmd

F32 = mybir.dt.float32
F32R = mybir.dt.float32r
BF16 = mybir.dt.bfloat16
AF = mybir.ActivationFunctionType
NPBF = ml_dtypes.bfloat16

B, N, C, H, D = 4, 2048, 1024, 16, 64
HG = 8            # heads per core
CG = HG * D       # 512: per-core c_out slice of q/k/v and of proj input
P = 128
E = D + 2         # 66: v columns + ones column + pad (4B-aligned bf16 slices)
MASK_VALUE = -65504.0
SCALE = float(D) ** -0.5

_prog_cache = {}


def _ceil_div(a, b):
    return (a + b - 1) // b


def _build(KU, use_qb):
    """Build the SPMD Bass program (same on all 8 cores) for KU kept keys."""
    KC = KU // P               # number of 128-token key chunks
    QB = N // 512              # 4 query blocks of 512

    nc = bacc.Bacc("TRN2", target_bir_lowering=False, debug=False, num_devices=8)
    xT_d = nc.declare_dram_parameter("xT", [C, N], BF16, isOutput=False)
    xpT_d = nc.declare_dram_parameter("xpT", [C, KU], BF16, isOutput=False)
    expb_d = nc.declare_dram_parameter("expbT", [KU, N], BF16, isOutput=False)
    wq_d = nc.declare_dram_parameter("wq", [P, 8 * CG], BF16, isOutput=False)
    wk_d = nc.declare_dram_parameter("wk", [P, 8 * CG], BF16, isOutput=False)
    wv_d = nc.declare_dram_parameter("wv", [P, 8 * CG], BF16, isOutput=False)
    wp_d = nc.declare_dram_parameter("wp", [P, 4 * C], BF16, isOutput=False)
    qb_d = nc.declare_dram_parameter("qb", [CG], F32, isOutput=False)
    vb_d = nc.declare_dram_parameter("vb", [1, CG], F32, isOutput=False)
    ones_d = nc.declare_dram_parameter("ones", [1, P], F32, isOutput=False)
    vones_d = nc.declare_dram_parameter("vones", [P, HG * E], BF16, isOutput=False)
    outp_d = nc.declare_dram_parameter("outp", [C, N], F32, isOutput=True)

    scr_d = nc.dram_tensor("rs_scratch", [16, 1024], F32)

    with ExitStack() as ctx:
        tc = ctx.enter_context(tile.TileContext(nc))
        persist = ctx.enter_context(tc.tile_pool(name="persist", bufs=1))
        const = ctx.enter_context(tc.tile_pool(name="const", bufs=1))

        ones1 = const.tile([1, P], F32R, name="ones1")
        nc.sync.dma_start(ones1[:], ones_d[:].bitcast(F32R))
        vb_t = const.tile([1, CG], F32R, name="vb_t")
        nc.sync.dma_start(vb_t[:], vb_d[:].bitcast(F32R))
        qb_t = const.tile([P, 4], F32, name="qb_t")
        for m in range(4):
            nc.sync.dma_start(
                qb_t[:, m : m + 1],
                qb_d[m * P : (m + 1) * P].rearrange("(p o) -> p o", o=1),
            )
        vo_t = const.tile([P, HG * E], BF16, name="vo_t")

        qTt = [persist.tile([P, N], BF16, name=f"qT{i}") for i in range(4)]
        kTt = [persist.tile([P, KU], BF16, name=f"kT{i}") for i in range(4)]
        vat = [persist.tile([P, HG * E], BF16, name=f"va{i}") for i in range(KC)]
        ott = [persist.tile([P, N], BF16, name=f"ot{i}") for i in range(4)]
        wp_t = persist.tile([P, 4 * C], BF16, name="wp_t")
        wq_t = persist.tile([P, 8 * CG], BF16, name="wq_t")
        wk_t = persist.tile([P, 8 * CG], BF16, name="wk_t")
        wv_t = persist.tile([P, 8 * CG], BF16, name="wv_t")
        xf = [persist.tile([P, N], BF16, name=f"xf{k}") for k in range(8)]
        xp = [persist.tile([P, KU], BF16, name=f"xp{k}") for k in range(8)]

        kblks = [(b0, min(512, KU - b0)) for b0 in range(0, KU, 512)]

        # ---- DMA issue order = compute priority ----
        for _j in range(8):
            nc.sync.dma_start(
                wk_t[:, _j * CG : (_j + 1) * CG], wk_d[:, _j * CG : (_j + 1) * CG]
            )
        half = KU // 2
        for k in range(8):
            nc.sync.dma_start(xp[k][:, 0:half], xpT_d[k * P : (k + 1) * P, 0:half])
            nc.sync.dma_start(
                xp[k][:, half:KU], xpT_d[k * P : (k + 1) * P, half:KU]
            )
        for _j in range(8):
            nc.sync.dma_start(
                wq_t[:, _j * CG : (_j + 1) * CG], wq_d[:, _j * CG : (_j + 1) * CG]
            )
        for k in range(8):
            nc.sync.dma_start(xf[k][:, 0:1024], xT_d[k * P : (k + 1) * P, 0:1024])
        for _j in range(8):
            nc.sync.dma_start(
                wv_t[:, _j * CG : (_j + 1) * CG], wv_d[:, _j * CG : (_j + 1) * CG]
            )
        nc.sync.dma_start(vo_t[:], vones_d[:])
        for k in range(8):
            nc.sync.dma_start(xf[k][:, 1024:N], xT_d[k * P : (k + 1) * P, 1024:N])
        for _j in range(8):
            nc.sync.dma_start(
                wp_t[:, _j * 512 : (_j + 1) * 512], wp_d[:, _j * 512 : (_j + 1) * 512]
            )

        with tc.tile_pool(name="bsb", bufs=KC + 2) as bpool, tc.tile_pool(
            name="pp", bufs=4
        ) as ppool, tc.tile_pool(name="ovp", bufs=3) as ovpool, tc.tile_pool(
            name="rsp", bufs=4
        ) as rpool, tc.tile_pool(name="oev2", bufs=3) as oev2, tc.tile_pool(
            name="bcp", bufs=2
        ) as bcpool, tc.tile_pool(
            name="pst", bufs=2, space="PSUM"
        ) as pst, tc.tile_pool(
            name="ppv", bufs=1, space="PSUM"
        ) as ppv, tc.tile_pool(
            name="fps", bufs=2, space="PSUM"
        ) as fps:

            # ---- emit units (each ~1.7-2.1us of PE work through fps) ----
            def emit_kT_block(m, i):
                b0, w = kblks[i]
                ps = fps.tile([P, 512], F32, name="ps_k", tag="fps")
                for kc8 in range(8):
                    lw = wk_t[:, kc8 * CG + m * P : kc8 * CG + (m + 1) * P]
                    nc.tensor.matmul(
                        ps[:, :w],
                        lhsT=lw,
                        rhs=xp[kc8][:, b0 : b0 + w],
                        start=(kc8 == 0),
                        stop=(kc8 == 7),
                    )
                nc.vector.tensor_copy(kTt[m][:, b0 : b0 + w], ps[:, :w])

            def emit_qT_block(m, nb):
                ps = fps.tile([P, 512], F32, name="ps_q", tag="fps")
                for kc8 in range(8):
                    lw = wq_t[:, kc8 * CG + m * P : kc8 * CG + (m + 1) * P]
                    nc.tensor.matmul(
                        ps[:],
                        lhsT=lw,
                        rhs=xf[kc8][:, nb * 512 : (nb + 1) * 512],
                        start=(kc8 == 0),
                        stop=(kc8 == 7),
                    )
                if use_qb:
                    nc.scalar.activation(
                        qTt[m][:, nb * 512 : (nb + 1) * 512],
                        ps[:],
                        AF.Identity,
                        bias=qb_t[:, m : m + 1],
                    )
                else:
                    nc.vector.tensor_copy(
                        qTt[m][:, nb * 512 : (nb + 1) * 512], ps[:]
                    )

            def emit_v_chunk(tm):
                psv = fps.tile([P, CG], F32, name="ps_v", tag="fps")
                for kc8 in range(8):
                    nc.tensor.matmul(
                        psv[:],
                        lhsT=xp[kc8][:, tm * P : (tm + 1) * P],
                        rhs=wv_t[:, kc8 * CG : (kc8 + 1) * CG],
                        start=(kc8 == 0),
                        stop=False,
                    )
                nc.tensor.matmul(
                    psv[:], lhsT=ones1[0:1, :], rhs=vb_t[0:1, :], start=False,
                    stop=True,
                )
                nc.vector.tensor_copy(vat[tm][:], vo_t[:])
                nc.vector.tensor_copy(
                    vat[tm][:].rearrange("p (h e) -> p h e", e=E)[:, :, 0:D],
                    psv[:].rearrange("p (h e) -> p h e", e=D),
                )

            def emit_proj_cq(cm, qs):
                ps = fps.tile([P, 512], F32, name="ps_p", tag="fps")
                for t4 in range(4):
                    lw = wp_t[:, t4 * C + cm * P : t4 * C + (cm + 1) * P]
                    nc.tensor.matmul(
                        ps[:],
                        lhsT=lw,
                        rhs=ott[t4][:, qs * 512 : (qs + 1) * 512],
                        start=(t4 == 0),
                        stop=(t4 == 3),
                    )
                osb = oev2.tile([P, 512], F32, name="o_sb", tag="osb")
                nc.vector.tensor_copy(osb[:], ps[:])
                nc.sync.dma_start(
                    outp_d[cm * P : (cm + 1) * P, qs * 512 : (qs + 1) * 512],
                    osb[:],
                )

            def emit_proj_wave(qs, psp):
                # 8 chains accumulated in t4-major waves: the only PE stall
                # is the first chain's t4=3 matmul (waits the last ott write)
                pss = [
                    psp.tile([P, 512], F32, name="ps_p3", tag="psp")
                    for _ in range(8)
                ]
                for t4 in range(4):
                    for cm in range(8):
                        lw = wp_t[:, t4 * C + cm * P : t4 * C + (cm + 1) * P]
                        nc.tensor.matmul(
                            pss[cm][:],
                            lhsT=lw,
                            rhs=ott[t4][:, qs * 512 : (qs + 1) * 512],
                            start=(t4 == 0),
                            stop=(t4 == 3),
                        )
                for cm in range(8):
                    osb = oev2.tile([P, 512], F32, name="o_sb3", tag="osb")
                    nc.scalar.activation(osb[:], pss[cm][:], AF.Copy)
                    nc.sync.dma_start(
                        outp_d[cm * P : (cm + 1) * P, qs * 512 : (qs + 1) * 512],
                        osb[:],
                    )

            def emit_dummy():
                # keeps the HAM activity window busy; no consumers
                ps = fps.tile([P, 512], F32, name="ps_d", tag="fps")
                nc.tensor.matmul(
                    ps[:], lhsT=wp_t[:, 0:P], rhs=ott[0][:, 0:512],
                    start=True, stop=True,
                )

            # ---- pre-attention GEMMs (kT/qT pair 0 + all v) ----
            for i in range(len(kblks)):
                emit_kT_block(0, i)
            for tm in range(KC):
                emit_v_chunk(tm)
            emit_qT_block(0, 0)
            emit_qT_block(0, 1)

            # ---- filler schedule: slot (qp, t) -> list of thunks ----
            def filler_for(qp, t):
                th = []
                if qp == 0:
                    if t < 3:
                        m = t + 1
                        for i in range(len(kblks)):
                            th.append(lambda m=m, i=i: emit_kT_block(m, i))
                        th.append(lambda m=m: emit_qT_block(m, 0))
                        th.append(lambda m=m: emit_qT_block(m, 1))
                    if t > 0:
                        # qp=1 half of the previous pair's qT, needed at (qp1, t-1)
                        th.append(lambda m=t: emit_qT_block(t, 2))
                        th.append(lambda m=t: emit_qT_block(t, 3))
                    if t == 3:
                        th.append(lambda: emit_qT_block(0, 2))
                        th.append(lambda: emit_qT_block(0, 3))
                        for _ in range(4):
                            th.append(emit_dummy)
                else:
                    for cm in (2 * t, 2 * t + 1):
                        for qs in range(2):
                            th.append(lambda cm=cm, qs=qs: emit_proj_cq(cm, qs))
                    for _ in range(2 + 2 * (t // 2)):
                        th.append(emit_dummy)
                return th

            for qp in range(QB // 2):
                q0 = qp * 1024
                btiles = []
                for kc in range(KC):
                    bt = bpool.tile([P, 1024], BF16, name="b_t", tag="bt")
                    nc.sync.dma_start(
                        bt[:], expb_d[kc * P : (kc + 1) * P, q0 : q0 + 1024]
                    )
                    btiles.append(bt)
                for t in range(4):
                    th = filler_for(qp, t)
                    nsteps = 2 * KC
                    sched = {}
                    for i, fn in enumerate(th):
                        step = min(nsteps - 1, (i * nsteps) // max(len(th), 1) + 1)
                        sched.setdefault(step, []).append(fn)
                    step = 0
                    for hh in range(2):
                        h = 2 * t + hh
                        po = hh * D
                        pv = ppv.tile([P, 1024], F32, name="pv_t", tag="pv")
                        for kc in range(KC):
                            stt = pst.tile([P, 1024], F32, name="st_t", tag="stt")
                            lw = kTt[t][po : po + D, kc * P : (kc + 1) * P]
                            for j in range(2):
                                nc.tensor.matmul(
                                    stt[:, j * 512 : (j + 1) * 512],
                                    lhsT=lw,
                                    rhs=qTt[t][
                                        po : po + D,
                                        q0 + j * 512 : q0 + (j + 1) * 512,
                                    ],
                                    start=True,
                                    stop=True,
                                )
                            pt = ppool.tile([P, 1024], BF16, name="p_t", tag="pt")
                            nc.scalar.activation(pt[:], stt[:], AF.Exp)
                            nc.vector.tensor_mul(pt[:], pt[:], btiles[kc][:])
                            lv = vat[kc][:, h * E : (h + 1) * E]
                            for j in range(2):
                                nc.tensor.matmul(
                                    pv[0:E, j * 512 : (j + 1) * 512],
                                    lhsT=lv,
                                    rhs=pt[:, j * 512 : (j + 1) * 512],
                                    start=(kc == 0),
                                    stop=(kc == KC - 1),
                                )
                            for fn in sched.get(step, []):
                                fn()
                            step += 1
                        # evacuate pv fast, normalize from the SBUF copy
                        it = qp * HG + h
                        ov = ovpool.tile([P, 1024], F32, name="ov_t", tag="ov")
                        nc.vector.tensor_copy(ov[0:E, :], pv[0:E, :])
                        rsw = rpool.tile([P, 8], F32, name="rsw_t", tag="rsw")
                        nc.sync.dma_start(rsw[:, :], ov[D : D + 1, :])
                        rsw2 = rpool.tile([P, 8], F32, name="rsw2_t", tag="rsw2")
                        nc.vector.reciprocal(rsw2[:, :], rsw[:, :])
                        nc.sync.dma_start(scr_d[it : it + 1, :], rsw2[:, :])
                        bcs = bcpool.tile([D, 1024], F32, name="bcs_t", tag="bcs")
                        row = scr_d[it : it + 1, :]
                        nc.gpsimd.dma_start(
                            bcs[:, :],
                            bass.AP(
                                tensor=row.tensor,
                                offset=row.offset,
                                ap=[[0, D], [1, 1024]],
                            ),
                        )
                        nc.gpsimd.tensor_mul(
                            ott[t][po : po + D, q0 : q0 + 1024],
                            ov[0:D, :],
                            bcs[:, :],
                        )

        # ---- projection, qp=1 half (own scope; attention pools closed) ----
        with tc.tile_pool(name="psp", bufs=8, space="PSUM") as psp, tc.tile_pool(
            name="oev3", bufs=4
        ) as oev2b:
            oev2 = oev2b
            for qs in range(2, 4):
                emit_proj_wave(qs, psp)
    nc.finalize()
    return nc


def kernel(
    x=None,
    attention_mask=None,
    attention_bias=None,
    qkv_w=None,
    q_bias=None,
    v_bias=None,
    proj_w=None,
    proj_b=None,
):
    x = np.ascontiguousarray(np.asarray(x, dtype=np.float32))
    mask = np.asarray(attention_mask).astype(bool)
    bias = np.asarray(attention_bias, dtype=np.float32)
    qkv_w = np.asarray(qkv_w, dtype=np.float32)
    q_bias = np.asarray(q_bias, dtype=np.float32)
    v_bias = np.asarray(v_bias, dtype=np.float32)
    proj_w = np.asarray(proj_w, dtype=np.float32)
    proj_b = np.asarray(proj_b, dtype=np.float32)

    assert x.shape == (B, N, C), x.shape

    # --- mask compaction: unmasked keys first, keep KU of them ---
    perms, us = [], []
    for b in range(B):
        perms.append(np.argsort(mask[b], kind="stable"))
        us.append(int((~mask[b]).sum()))
    KU = min(N, max(P, _ceil_div(max(us), P) * P))
    use_qb = bool(np.any(q_bias))

    key = (KU, use_qb)
    if key not in _prog_cache:
        _prog_cache[key] = _build(KU, use_qb)
    nc = _prog_cache[key]

    ones_h = np.ones((1, P), dtype=np.float32)
    vones_h = np.zeros((P, HG * E), dtype=NPBF)
    vones_h.reshape(P, HG, E)[:, :, D] = 1.0
    mv = np.float32(MASK_VALUE)

    per_b = []
    for b in range(B):
        perm = perms[b][:KU]
        xT = np.ascontiguousarray(x[b].T.astype(NPBF))
        xpT = np.ascontiguousarray(x[b][perm].T.astype(NPBF))
        biasT = bias[b].T[perm] + np.where(mask[b][perm], mv, np.float32(0.0))[:, None]
        expbT = np.ascontiguousarray(np.exp(biasT, dtype=np.float32).astype(NPBF))
        per_b.append((xT, xpT, expbT))

    per_g = []
    for g in range(2):
        sl = slice(g * CG, (g + 1) * CG)

        def tile_w(wT, ncols):  # [C_in, ncols] -> [128, (C_in//128)*ncols]
            return np.ascontiguousarray(
                wT.reshape(wT.shape[0] // P, P, ncols)
                .transpose(1, 0, 2)
                .reshape(P, -1)
                .astype(NPBF)
            )

        wq = tile_w((qkv_w[sl, :] * np.float32(SCALE)).T.astype(np.float32), CG)
        wk = tile_w(np.ascontiguousarray(qkv_w[C + g * CG : C + (g + 1) * CG, :].T), CG)
        wv = tile_w(
            np.ascontiguousarray(qkv_w[2 * C + g * CG : 2 * C + (g + 1) * CG, :].T), CG
        )
        wp = tile_w(np.ascontiguousarray(proj_w[:, sl].T), C)
        qb = np.ascontiguousarray(q_bias[sl] * np.float32(SCALE))
        vb = np.ascontiguousarray(v_bias[sl][None, :])
        per_g.append((wq, wk, wv, wp, qb, vb))

    in_maps = []
    for c in range(8):
        b, g = c // 2, c % 2
        xT, xpT, expbT = per_b[b]
        wq, wk, wv, wp, qb, vb = per_g[g]
        in_maps.append(
            {
                "xT": xT,
                "xpT": xpT,
                "expbT": expbT,
                "wq": wq,
                "wk": wk,
                "wv": wv,
                "wp": wp,
                "qb": qb,
                "vb": vb,
                "ones": ones_h,
                "vones": vones_h,
            }
        )

    trace = bool(int(os.environ.get("KBENCH_TRACE", "0")))
    kw = {}
    if trace:
        kw = dict(
            trace=True,
            trace_cores=[
                int(t) for t in os.environ.get("KBENCH_TRACE_CORES", "0").split(",")
            ],
        )
    res = run_bass_kernel_spmd(nc, in_maps, list(range(8)), **kw)
    if trace:
        kernel.last_exec_ns = res.exec_time_ns
        kernel.last_result = res

    out = np.empty((B, N, C), dtype=np.float32)
    for b in range(B):
        outT = res.results[2 * b]["outp"] + res.results[2 * b + 1]["outp"]
        out[b] = outT.T
        out[b] += proj_b[None, :]
    return out


kernel.last_exec_ns = None
kernel.last_result = None
